# revision 14
# baseline (speedup 1.0000x reference)
"""SupJSD / ContrastiveLossPlus loss kernel for 8 Trainium2 NeuronCores.

Math: p_i = x_i/||x_i||, and with ln p = ln x - 0.5*ln(ss_i) (ss = sum x^2)
the loss needs only three per-class matrices, all PE matmuls:
  A_x[c,d] = sum_i 1hot*s16*x        (16*S_c, S_c = class prob sums)
  A_z[c,d] = sum_i 1hot*s16*z        (z = x*ln x)
  B[c,d]   = sum_i 1hot*(s16*lnss)*x
where sum_{i in c} sum_d p ln p = (sum_d A_z - 0.5*sum_d B)/16.

Per 16-tile group (128x4096 bf16), software-pipelined one group deep:
  BIG(g) : DMA x + fp8 one-hot labels; ACT batched Square(x)->q and
           Ln(x)->lnx; row sums of q via fold-chain (first fold on the
           otherwise-idle GpSimd engine, rest on DVE); DVE z = x*lnx
           into the interleaved [x|z] tile.
  FIN(g) : ACT tiny Ln/Exp for s16 = 16/sqrt(ss); DVE broadcast-mult
           stationary builds into 128-column-padded tiles (cols 80:127
           garbage -> psum rows the host ignores; 128-wide stationary
           enables the PE fast-weight-load path); per tile one 512-col
           matmul amat^T @ [x|z] + one 256-col bmat^T @ x.
One pinned ln+exp+square activation table load. Host combines the
[80,768] per-core accumulators in float64.
"""

import numpy as np

N_CORES = 8
N, D, C = 65536, 256, 80
CP = 128                      # padded stationary width (FWL needs 128)
R = 3 * N // N_CORES          # rows per core = 24576
T = R // 128                  # tiles per core = 192
G = 16                        # tiles per group
NG = T // G                   # groups = 12
LOG16 = float(np.log(16.0))
POOL_FOLD1 = True             # first q-fold on GpSimd instead of DVE

_cache = {}


def _build_nc():
    from contextlib import ExitStack

    import concourse.tile as tile
    from concourse import bacc, mybir

    F32 = mybir.dt.float32
    BF16 = mybir.dt.bfloat16
    FP8 = mybir.dt.float8e4
    A = mybir.AluOpType
    ACTF = mybir.ActivationFunctionType
    AX = mybir.AxisListType

    nc = bacc.Bacc("TRN2", target_bir_lowering=False, debug=False,
                   num_devices=N_CORES)
    # Partition-major: xin[p, g*4096 + j*256 + c] = x[(g*16+j)*128 + p, c]
    xin = nc.dram_tensor("xin", [128, T * 256], BF16, kind="ExternalInput").ap()
    # One-hot labels, tile-major fp8: ohin[p, g*G*C + j*C + c]
    ohin = nc.dram_tensor("ohin", [128, T * C], FP8,
                          kind="ExternalInput").ap()
    out = nc.dram_tensor("acc", [C, 768], F32, kind="ExternalOutput").ap()

    with tile.TileContext(nc) as tc, ExitStack() as ctx:
        cpool = ctx.enter_context(tc.tile_pool(name="consts", bufs=1))
        xzpool = ctx.enter_context(tc.tile_pool(name="xz", bufs=4))
        zpool = ctx.enter_context(tc.tile_pool(name="z", bufs=3))
        ohpool = ctx.enter_context(tc.tile_pool(name="oh", bufs=4))
        qpool = ctx.enter_context(tc.tile_pool(name="q", bufs=2))
        fpool = ctx.enter_context(tc.tile_pool(name="fold", bufs=2))
        lpool = ctx.enter_context(tc.tile_pool(name="lnx", bufs=2))
        spool = ctx.enter_context(tc.tile_pool(name="small", bufs=3))
        mpool = ctx.enter_context(tc.tile_pool(name="mats", bufs=2))
        opool = ctx.enter_context(tc.tile_pool(name="out", bufs=1))
        pspools = [ctx.enter_context(
            tc.tile_pool(name=f"ps{i}", bufs=1, space="PSUM"))
            for i in range(6)]

        c_tiny = cpool.tile([128, 1], F32)
        nc.vector.memset(c_tiny[:], 1e-30)
        c_ln16 = cpool.tile([128, 1], F32)
        nc.vector.memset(c_ln16[:], LOG16)

        # Pin the ln+exp+square table once: avoids per-group table thrash.
        nc.scalar.add_instruction(mybir.InstLoadActFuncSet(
            name=nc.get_next_instruction_name(), act_func_set_id=6,
            ins=[], outs=[]))

        psX = [pspools[i].tile([CP, 256], F32, name=f"psx{i}")
               for i in range(2)]
        psZ = [pspools[2 + i].tile([CP, 256], F32, name=f"psz{i}")
               for i in range(2)]
        psB = [pspools[4 + i].tile([CP, 256], F32, name=f"psb{i}")
               for i in range(2)]

        state = {}

        def dma_stage(g):
            xg = xzpool.tile([128, G * 256], BF16, tag="x")
            nc.sync.dma_start(xg[:], xin[:, g * G * 256:(g + 1) * G * 256])
            ohg = ohpool.tile([128, G, C], FP8, tag="oh")
            nc.sync.dma_start(ohg[:], ohin[:, g * G * C:(g + 1) * G * C]
                              .rearrange("p (t c) -> p t c", t=G))
            state[g] = {"x": xg, "oh": ohg}

        def big_stage(g):
            st = state[g]
            xg = st["x"]
            xv = xg[:].rearrange("p (t c) -> p t c", t=G)
            qg = qpool.tile([128, G * 256], BF16, tag="q")
            nc.scalar.activation(qg[:].rearrange("p (t c) -> p t c", t=G),
                                 xv, ACTF.Square)
            lnxg = lpool.tile([128, G * 256], BF16, tag="lnx")
            nc.scalar.activation(lnxg[:].rearrange("p (t c) -> p t c", t=G),
                                 xv, ACTF.Ln, bias=c_tiny[:])

            zg = zpool.tile([128, G * 256], BF16, tag="z")
            nc.vector.tensor_tensor(zg[:], xg[:], lnxg[:], A.mult)
            st["z"] = zg

            # fold-chain row sums of q: 256->128->64->32, then reduce
            f1 = fpool.tile([128, G, 128], BF16, tag="f1")
            q3 = qg[:].rearrange("p (t c) -> p t c", t=G)
            eng1 = nc.gpsimd if POOL_FOLD1 else nc.vector
            eng1.tensor_tensor(f1[:], q3[:, :, 0:128], q3[:, :, 128:256],
                               A.add)
            eng1.tensor_tensor(f1[:, :, 0:64], f1[:, :, 0:64],
                               f1[:, :, 64:128], A.add)
            eng1.tensor_tensor(f1[:, :, 0:32], f1[:, :, 0:32],
                               f1[:, :, 32:64], A.add)
            ssg = spool.tile([128, G], F32, tag="ss")
            nc.vector.tensor_reduce(ssg[:], f1[:, :, 0:32], AX.X, A.add)
            st["ss"] = ssg

        def fin_stage(g):
            st = state.pop(g)
            xg, zg, ohg, ssg = st["x"], st["z"], st["oh"], st["ss"]
            x3 = xg[:].rearrange("p (t c) -> p t c", t=G)
            z3 = zg[:].rearrange("p (t c) -> p t c", t=G)
            lnssg = spool.tile([128, G], F32, tag="lnss")
            nc.scalar.activation(lnssg[:], ssg[:], ACTF.Ln)
            s16b = spool.tile([128, G], BF16, tag="s16b")
            nc.scalar.activation(s16b[:], lnssg[:], ACTF.Exp,
                                 bias=c_ln16[:], scale=-0.5)
            blb = spool.tile([128, G], BF16, tag="blb")
            nc.vector.tensor_tensor(blb[:], s16b[:], lnssg[:], A.mult)

            # padded stationaries; cols C:CP never written (psum rows
            # 80:127 accumulate garbage the host never reads)
            amat = mpool.tile([128, G, CP], BF16, tag="amat")
            nc.vector.tensor_tensor(
                amat[:, :, 0:C], ohg[:],
                s16b[:].rearrange("p (t o) -> p t o", o=1)
                .broadcast_to([128, G, C]), A.mult)
            bmat = mpool.tile([128, G, CP], BF16, tag="bmat")
            nc.vector.tensor_tensor(
                bmat[:, :, 0:C], ohg[:],
                blb[:].rearrange("p (t o) -> p t o", o=1)
                .broadcast_to([128, G, C]), A.mult)

            for j in range(G):
                k = g * G + j
                first, last = (k < 2), (k >= T - 2)
                nc.tensor.matmul(psX[j % 2][:], amat[:, j, :],
                                 x3[:, j, :], start=first, stop=last)
            for j in range(G):
                k = g * G + j
                first, last = (k < 2), (k >= T - 2)
                nc.tensor.matmul(psZ[j % 2][:], amat[:, j, :],
                                 z3[:, j, :], start=first, stop=last)
            for j in range(G):
                k = g * G + j
                first, last = (k < 2), (k >= T - 2)
                nc.tensor.matmul(psB[j % 2][:], bmat[:, j, :],
                                 x3[:, j, :], start=first, stop=last)

        dma_stage(0)
        dma_stage(1)
        for g in range(NG):
            if g > 0:
                fin_stage(g - 1)
            if g + 2 < NG:
                dma_stage(g + 2)
            big_stage(g)
        fin_stage(NG - 1)

        accs = opool.tile([C, 768], F32)
        for i, (p0, p1) in enumerate([(psX[0], psX[1]), (psZ[0], psZ[1]),
                                      (psB[0], psB[1])]):
            sl = slice(i * 256, (i + 1) * 256)
            nc.vector.tensor_copy(accs[:, sl], p0[0:C, :])
            nc.vector.tensor_tensor(accs[:, sl], accs[:, sl], p1[0:C, :],
                                    A.add)
        nc.sync.dma_start(out[:], accs[:])
    nc.compile()
    return nc


def _get_nc():
    if "nc" not in _cache:
        _cache["nc"] = _build_nc()
    return _cache["nc"]


def kernel(logits_clean, logits_aug1, logits_aug2, labels):
    import os

    import ml_dtypes
    from concourse import mybir
    from concourse.bass_utils import run_bass_kernel_spmd

    BF = ml_dtypes.bfloat16
    F8 = mybir.dt.np(mybir.dt.float8e4)
    x3 = np.concatenate(
        [np.asarray(logits_clean, dtype=np.float32),
         np.asarray(logits_aug1, dtype=np.float32),
         np.asarray(logits_aug2, dtype=np.float32)], axis=0)
    lab1 = np.asarray(labels).astype(np.int64)
    lab3 = np.concatenate([lab1, lab1, lab1])

    cls_ar = np.arange(C, dtype=np.int64)
    in_maps = []
    for c in range(N_CORES):
        sl = slice(c * R, (c + 1) * R)
        xc = x3[sl].astype(BF).reshape(T, 128, D).transpose(1, 0, 2)
        # one-hot [128, T, C]: oh[p, k, c] = (lab[k*128+p] == c)
        L = lab3[sl].reshape(T, 128)
        oh = (L.T[:, :, None] == cls_ar[None, None, :]).astype(F8)
        in_maps.append({
            "xin": np.ascontiguousarray(xc).reshape(128, T * D),
            "ohin": np.ascontiguousarray(oh).reshape(128, T * C),
        })

    nc = _get_nc()
    trace = bool(int(os.environ.get("KERNEL_TRACE", "0")))
    kw = {}
    if trace:
        kw = dict(trace=True, tmpdir=os.environ.get("KERNEL_TRACE_DIR"))
    br = run_bass_kernel_spmd(nc, in_maps, list(range(N_CORES)), **kw)
    _cache["last_results"] = br

    acc = np.zeros((C, 768), np.float64)
    for c in range(N_CORES):
        acc += br.results[c]["acc"].astype(np.float64)

    seg = acc[:, 0:D] / 16.0                      # S_c per dim
    E = (acc[:, D:2 * D].sum(1) - 0.5 * acc[:, 2 * D:3 * D].sum(1)) / 16.0
    counts = np.bincount(lab3, minlength=C).astype(np.float64)
    mix = seg / np.maximum(counts, 1.0)[:, None]
    lm = np.log(np.maximum(mix, 1e-7))
    num = E - (seg * lm).sum(1)
    loss = np.where(counts > 0, num / np.maximum(counts, 1.0), 0.0).sum() / D
    return np.float32(0.01 * loss)


# revision 15
# speedup vs baseline: 1.1132x; 1.1132x over previous
"""SupJSD / ContrastiveLossPlus loss kernel for 8 Trainium2 NeuronCores.

Math: p_i = x_i/||x_i||, and with ln p = ln x - 0.5*ln(ss_i) (ss = sum x^2)
the loss needs only three per-class matrices, all PE matmuls:
  A_x[c,d] = sum_i 1hot*s16*x        (16*S_c, S_c = class prob sums)
  A_z[c,d] = sum_i 1hot*s16*z        (z = x*ln x)
  B[c,d]   = sum_i 1hot*(s16*lnss)*x
where sum_{i in c} sum_d p ln p = (sum_d A_z - 0.5*sum_d B)/16.

Per 16-tile group (128x4096 bf16), software-pipelined one group deep:
  BIG(g) : DMA x + fp8 one-hot labels; ACT batched Square(x)->q and
           Ln(x)->lnx; row sums of q via fold-chain (first fold on the
           otherwise-idle GpSimd engine, rest on DVE); DVE z = x*lnx
           into the interleaved [x|z] tile.
  FIN(g) : ACT tiny Ln/Exp for s16 = 16/sqrt(ss); DVE broadcast-mult
           stationary builds into 128-column-padded tiles (cols 80:127
           garbage -> psum rows the host ignores; 128-wide stationary
           enables the PE fast-weight-load path); per tile one 512-col
           matmul amat^T @ [x|z] + one 256-col bmat^T @ x.
One pinned ln+exp+square activation table load. Host combines the
[80,768] per-core accumulators in float64.
"""

import numpy as np

N_CORES = 8
N, D, C = 65536, 256, 80
CP = 128                      # padded stationary width (FWL needs 128)
R = 3 * N // N_CORES          # rows per core = 24576
T = R // 128                  # tiles per core = 192
G = 16                        # tiles per group
NG = T // G                   # groups = 12
LOG16 = float(np.log(16.0))
POOL_FOLD1 = True             # first q-fold on GpSimd instead of DVE

_cache = {}


def _build_nc():
    from contextlib import ExitStack

    import concourse.tile as tile
    from concourse import bacc, mybir

    F32 = mybir.dt.float32
    BF16 = mybir.dt.bfloat16
    FP8 = mybir.dt.float8e4
    A = mybir.AluOpType
    ACTF = mybir.ActivationFunctionType
    AX = mybir.AxisListType

    nc = bacc.Bacc("TRN2", target_bir_lowering=False, debug=False,
                   num_devices=N_CORES)
    # Partition-major: xin[p, g*4096 + j*256 + c] = x[(g*16+j)*128 + p, c]
    xin = nc.dram_tensor("xin", [128, T * 256], BF16, kind="ExternalInput").ap()
    # One-hot labels, tile-major fp8: ohin[p, g*G*C + j*C + c]
    ohin = nc.dram_tensor("ohin", [128, T * C], FP8,
                          kind="ExternalInput").ap()
    out = nc.dram_tensor("acc", [C, 768], F32, kind="ExternalOutput").ap()

    with tile.TileContext(nc) as tc, ExitStack() as ctx:
        cpool = ctx.enter_context(tc.tile_pool(name="consts", bufs=1))
        xzpool = ctx.enter_context(tc.tile_pool(name="xz", bufs=4))
        zpool = ctx.enter_context(tc.tile_pool(name="z", bufs=3))
        ohpool = ctx.enter_context(tc.tile_pool(name="oh", bufs=4))
        qpool = ctx.enter_context(tc.tile_pool(name="q", bufs=2))
        fpool = ctx.enter_context(tc.tile_pool(name="fold", bufs=2))
        lpool = ctx.enter_context(tc.tile_pool(name="lnx", bufs=2))
        spool = ctx.enter_context(tc.tile_pool(name="small", bufs=3))
        mpool = ctx.enter_context(tc.tile_pool(name="mats", bufs=2))
        opool = ctx.enter_context(tc.tile_pool(name="out", bufs=1))
        pspools = [ctx.enter_context(
            tc.tile_pool(name=f"ps{i}", bufs=1, space="PSUM"))
            for i in range(6)]

        c_tiny = cpool.tile([128, 1], F32)
        nc.vector.memset(c_tiny[:], 1e-30)
        c_ln16 = cpool.tile([128, 1], F32)
        nc.vector.memset(c_ln16[:], LOG16)

        # Pin the ln+exp+square table once: avoids per-group table thrash.
        nc.scalar.add_instruction(mybir.InstLoadActFuncSet(
            name=nc.get_next_instruction_name(), act_func_set_id=6,
            ins=[], outs=[]))

        psX = [pspools[i].tile([CP, 256], F32, name=f"psx{i}")
               for i in range(2)]
        psZ = [pspools[2 + i].tile([CP, 256], F32, name=f"psz{i}")
               for i in range(2)]
        psB = [pspools[4 + i].tile([CP, 256], F32, name=f"psb{i}")
               for i in range(2)]

        state = {}

        def dma_stage(g):
            xg = xzpool.tile([128, G * 256], BF16, tag="x")
            nc.sync.dma_start(xg[:], xin[:, g * G * 256:(g + 1) * G * 256])
            ohg = ohpool.tile([128, G, C], FP8, tag="oh")
            nc.sync.dma_start(ohg[:], ohin[:, g * G * C:(g + 1) * G * C]
                              .rearrange("p (t c) -> p t c", t=G))
            state[g] = {"x": xg, "oh": ohg}

        def big_stage(g):
            st = state[g]
            xg = st["x"]
            xv = xg[:].rearrange("p (t c) -> p t c", t=G)
            qg = qpool.tile([128, G * 256], BF16, tag="q")
            nc.scalar.activation(qg[:].rearrange("p (t c) -> p t c", t=G),
                                 xv, ACTF.Square)
            lnxg = lpool.tile([128, G * 256], BF16, tag="lnx")
            nc.scalar.activation(lnxg[:].rearrange("p (t c) -> p t c", t=G),
                                 xv, ACTF.Ln, bias=c_tiny[:])

            zg = zpool.tile([128, G * 256], BF16, tag="z")
            nc.vector.tensor_tensor(zg[:], xg[:], lnxg[:], A.mult)
            st["z"] = zg

            # fold-chain row sums of q: 256->128->64->32, then reduce
            f1 = fpool.tile([128, G, 128], BF16, tag="f1")
            q3 = qg[:].rearrange("p (t c) -> p t c", t=G)
            eng1 = nc.gpsimd if POOL_FOLD1 else nc.vector
            eng1.tensor_tensor(f1[:], q3[:, :, 0:128], q3[:, :, 128:256],
                               A.add)
            nc.vector.tensor_tensor(f1[:, :, 0:64], f1[:, :, 0:64],
                                    f1[:, :, 64:128], A.add)
            nc.vector.tensor_tensor(f1[:, :, 0:32], f1[:, :, 0:32],
                                    f1[:, :, 32:64], A.add)
            ssg = spool.tile([128, G], F32, tag="ss")
            nc.vector.tensor_reduce(ssg[:], f1[:, :, 0:32], AX.X, A.add)
            st["ss"] = ssg

        def fin_stage(g):
            st = state.pop(g)
            xg, zg, ohg, ssg = st["x"], st["z"], st["oh"], st["ss"]
            x3 = xg[:].rearrange("p (t c) -> p t c", t=G)
            z3 = zg[:].rearrange("p (t c) -> p t c", t=G)
            lnssg = spool.tile([128, G], F32, tag="lnss")
            nc.scalar.activation(lnssg[:], ssg[:], ACTF.Ln)
            s16b = spool.tile([128, G], BF16, tag="s16b")
            nc.scalar.activation(s16b[:], lnssg[:], ACTF.Exp,
                                 bias=c_ln16[:], scale=-0.5)
            blb = spool.tile([128, G], BF16, tag="blb")
            nc.vector.tensor_tensor(blb[:], s16b[:], lnssg[:], A.mult)

            # padded stationaries; cols C:CP never written (psum rows
            # 80:127 accumulate garbage the host never reads)
            amat = mpool.tile([128, G, CP], BF16, tag="amat")
            nc.vector.tensor_tensor(
                amat[:, :, 0:C], ohg[:],
                s16b[:].rearrange("p (t o) -> p t o", o=1)
                .broadcast_to([128, G, C]), A.mult)
            bmat = mpool.tile([128, G, CP], BF16, tag="bmat")
            nc.vector.tensor_tensor(
                bmat[:, :, 0:C], ohg[:],
                blb[:].rearrange("p (t o) -> p t o", o=1)
                .broadcast_to([128, G, C]), A.mult)

            for j in range(G):
                k = g * G + j
                first, last = (k < 2), (k >= T - 2)
                nc.tensor.matmul(psX[j % 2][:], amat[:, j, :],
                                 x3[:, j, :], start=first, stop=last)
            for j in range(G):
                k = g * G + j
                first, last = (k < 2), (k >= T - 2)
                nc.tensor.matmul(psZ[j % 2][:], amat[:, j, :],
                                 z3[:, j, :], start=first, stop=last)
            for j in range(G):
                k = g * G + j
                first, last = (k < 2), (k >= T - 2)
                nc.tensor.matmul(psB[j % 2][:], bmat[:, j, :],
                                 x3[:, j, :], start=first, stop=last)

        dma_stage(0)
        dma_stage(1)
        for g in range(NG):
            if g > 0:
                fin_stage(g - 1)
            if g + 2 < NG:
                dma_stage(g + 2)
            big_stage(g)
        fin_stage(NG - 1)

        accs = opool.tile([C, 768], F32)
        for i, (p0, p1) in enumerate([(psX[0], psX[1]), (psZ[0], psZ[1]),
                                      (psB[0], psB[1])]):
            sl = slice(i * 256, (i + 1) * 256)
            nc.vector.tensor_copy(accs[:, sl], p0[0:C, :])
            nc.vector.tensor_tensor(accs[:, sl], accs[:, sl], p1[0:C, :],
                                    A.add)
        nc.sync.dma_start(out[:], accs[:])
    nc.compile()
    return nc


def _get_nc():
    if "nc" not in _cache:
        _cache["nc"] = _build_nc()
    return _cache["nc"]


def kernel(logits_clean, logits_aug1, logits_aug2, labels):
    import os

    import ml_dtypes
    from concourse import mybir
    from concourse.bass_utils import run_bass_kernel_spmd

    BF = ml_dtypes.bfloat16
    F8 = mybir.dt.np(mybir.dt.float8e4)
    x3 = np.concatenate(
        [np.asarray(logits_clean, dtype=np.float32),
         np.asarray(logits_aug1, dtype=np.float32),
         np.asarray(logits_aug2, dtype=np.float32)], axis=0)
    lab1 = np.asarray(labels).astype(np.int64)
    lab3 = np.concatenate([lab1, lab1, lab1])

    cls_ar = np.arange(C, dtype=np.int64)
    in_maps = []
    for c in range(N_CORES):
        sl = slice(c * R, (c + 1) * R)
        xc = x3[sl].astype(BF).reshape(T, 128, D).transpose(1, 0, 2)
        # one-hot [128, T, C]: oh[p, k, c] = (lab[k*128+p] == c)
        L = lab3[sl].reshape(T, 128)
        oh = (L.T[:, :, None] == cls_ar[None, None, :]).astype(F8)
        in_maps.append({
            "xin": np.ascontiguousarray(xc).reshape(128, T * D),
            "ohin": np.ascontiguousarray(oh).reshape(128, T * C),
        })

    nc = _get_nc()
    trace = bool(int(os.environ.get("KERNEL_TRACE", "0")))
    kw = {}
    if trace:
        kw = dict(trace=True, tmpdir=os.environ.get("KERNEL_TRACE_DIR"))
    br = run_bass_kernel_spmd(nc, in_maps, list(range(N_CORES)), **kw)
    _cache["last_results"] = br

    acc = np.zeros((C, 768), np.float64)
    for c in range(N_CORES):
        acc += br.results[c]["acc"].astype(np.float64)

    seg = acc[:, 0:D] / 16.0                      # S_c per dim
    E = (acc[:, D:2 * D].sum(1) - 0.5 * acc[:, 2 * D:3 * D].sum(1)) / 16.0
    counts = np.bincount(lab3, minlength=C).astype(np.float64)
    mix = seg / np.maximum(counts, 1.0)[:, None]
    lm = np.log(np.maximum(mix, 1e-7))
    num = E - (seg * lm).sum(1)
    loss = np.where(counts > 0, num / np.maximum(counts, 1.0), 0.0).sum() / D
    return np.float32(0.01 * loss)
